# revision 8
# baseline (speedup 1.0000x reference)
"""Trainium2 Bass kernel for ArcShapeRadiusConfigVisibleNeighDist.

For each pedestrian i (N=8192):
  heading u_i = normalize(pos_i - past_i)
  over all j: dist_ij = |pos_j - pos_i|, visible iff angle(pos_j-pos_i, u_i)
  in [-35deg, 35deg) and j != i. Output = affine(clip(mean visible dist)).

Reformulation (no atan2): visible <=> dot/c > dist. sq and dot/c are
K-small matmuls on the TensorEngine with fp16 hi/lo split features. The
4 matmuls per chunk go to the 4 PE row-quads (tile_position 0/32/64/96)
so they stream concurrently. (Note: a DVE op may read at most ONE PSUM
operand, so the select must compare against SBUF dist, not PSUM sq.)

Per 128-query x 1024-j chunk:
  PE:   g1 = sq+eps, g2 = dot/c   (PSUM fp32, 2x512 each)
  ACT:  dist = sqrt(g1) -> fp16 SBUF
  DVE:  sd = select(g2 > dist, dist, 0) -> fp16, accum_out -> s (custom)
Per 2048-j pair of chunks:
  count: cnt = sum(sd > 0): DVE tensor_scalar+accum or ACT Sign+accum,
  per CNT_SCHED to balance engine load.
Epilogue (host): r = clip(slope*(s/max(cnt,1)) + off, 0.5, 4.0); indexes sel.

Sharding: core k owns queries [k*1024, (k+1)*1024), full j set.
"""

import numpy as np

import concourse.bass as bass
import concourse.bacc as bacc
import concourse.mybir as mybir
import concourse.tile as tile
from contextlib import ExitStack
from concourse.alu_op_type import AluOpType
from concourse.bass_utils import run_bass_kernel_spmd
from concourse.dve_uop import DveOpSpec
import concourse.dve_ops as dvo
from concourse.dve_ops import Spec, Src0, Src1, Zero, select, lower, has_src1
from concourse.dve_ops import AluOp as SAluOp
from concourse.dve_spec import sq as spec_sq

N = 8192
NCORES = 8
Q = N // NCORES            # 1024 queries per core
ITILES = Q // 128          # 8 partition tiles of queries
JCHUNK = 1024
JPAIR = 2 * JCHUNK         # 2048: sqrt/count granularity
NJP = N // JPAIR           # 4 j-pairs per i-tile
NJC = N // JCHUNK          # 8 j-chunks per i-tile
EPS = 0.005                # sq guard: keeps diag excluded, sqrt input > 0
COS_HALF = float(np.cos(70.0 * np.pi / 180.0 / 2.0))
MIN_R, MAX_R = 0.5, 4.0
MIN_D, MAX_D = 0.2, 5.0
SLOPE = (MAX_R - MIN_R) / (MAX_D - MIN_D)
OFFS = MIN_R - MIN_D * SLOPE

F32 = mybir.dt.float32
FP16 = mybir.dt.float16
FP8 = mybir.dt.float8e4
ACTF = mybir.ActivationFunctionType
_F16 = np.float16

# count-pass engine per pair (it * NJP + p): 'v' DVE tensor_scalar+accum,
# 'a' ACT Sign+accum. Tuned so DVE ~= ACT total busy.
N_V = 16                   # of ITILES*NJP = 32 pairs
CNT_SCHED = [('v' if (i * 19) % 32 < N_V else 'a') for i in range(ITILES * NJP)]
JF_SPLIT = 4               # jf DMA'd as column tiles so first matmuls start early


def register_masked_sd():
    """Legacy v1 op (kept for probe compatibility): out = select(in0 > in1,
    in1, 0), accum_out = sum(out)."""
    name = "MASKED_SD_ANT"
    if name in dvo._SUB_OPCODE_FOR_NAME:
        return getattr(dvo, name)

    def _ref(in0, in1, s0, s1, imm2):
        b = np.where(in0.astype(np.float32) > in1, in1, 0.0).astype(np.float32)
        return b, b.reshape(b.shape[0], -1).sum(axis=-1, keepdims=True)

    spec = Spec(body=select(Src0 > Src1, Src1, Zero), accum=SAluOp.ADD,
                reference=_ref)
    return _register(name, spec)


def register_masked_sq():
    """out = select((in0 > 0) & (in0^2 > in1), in1, 0); no accum.
    in0 = dot/c, in1 = sq + eps: squared-domain visibility select."""
    name = "MASKED_SQ_ANT"
    if name in dvo._SUB_OPCODE_FOR_NAME:
        return getattr(dvo, name)

    def _ref(in0, in1, s0, s1, imm2):
        a = in0.astype(np.float32)
        b = in1.astype(np.float32)
        return np.where((a > 0) & (a * a > b), b, 0.0).astype(np.float32)

    spec = Spec(body=select((Src0 > Zero) & (spec_sq(Src0) > Src1), Src1, Zero),
                reference=_ref)
    return _register(name, spec)


def _register(name, spec):
    """Runtime-register a custom DVE op. The per-NEFF uop table is generated
    from OPS, so appending at runtime is sufficient."""
    row = max(dvo._SUB_OPCODE_FOR_NAME.values()) + 1
    assert row < 0x20
    dvo._SUB_OPCODE_FOR_NAME[name] = row
    op = dvo.DveOp(name, spec, subdim=False, uops_sha={})
    for ver in ("v3", "v4"):
        s = DveOpSpec(name=name, opcode=row, uops=lower(spec, ver=ver),
                      rd1_en=has_src1(spec))
        op.uops_sha[ver] = s.sha(ver)
    dvo.OPS.append(op)
    dvo.CUSTOM_DVE_SPECS[name] = spec
    setattr(dvo, name, op)
    return op


def _split(x):
    """Split f64 array into fp16 hi + fp16 lo (as f64 of exact fp16 values)."""
    h = x.astype(_F16).astype(np.float64)
    l = (x - h).astype(_F16).astype(np.float64)
    return h, l


def _build_graph():
    masked_sd = register_masked_sd()
    nc = bacc.Bacc("TRN2", target_bir_lowering=False, debug=False,
                   num_devices=NCORES)
    # feature rows: G1 at partitions 0-9 and 32-41 (quads 0,1); G2 at
    # 64-71 and 96-103 (quads 2,3) -> 4 concurrent matmul streams.
    qf1_d = nc.dram_tensor("qf1", [10, Q], FP16, kind="ExternalInput")
    qf2_d = nc.dram_tensor("qf2", [8, Q], FP16, kind="ExternalInput")
    jf1_d = nc.dram_tensor("jf1", [10, N], FP16, kind="ExternalInput")
    jf2_d = nc.dram_tensor("jf2", [8, N], FP16, kind="ExternalInput")
    os_d = nc.dram_tensor("out_s", [128, ITILES * NJC], F32,
                          kind="ExternalOutput")
    oc_d = nc.dram_tensor("out_c", [128, ITILES * NJP], F32,
                          kind="ExternalOutput")

    with tile.TileContext(nc) as tc, ExitStack() as ctx:
        singles = ctx.enter_context(tc.tile_pool(name="singles", bufs=1))
        psum = ctx.enter_context(tc.tile_pool(name="psum", bufs=2, space="PSUM"))
        dsp = ctx.enter_context(tc.tile_pool(name="dsp", bufs=3))
        sdp = ctx.enter_context(tc.tile_pool(name="sdp", bufs=3))
        scr = ctx.enter_context(tc.tile_pool(name="scr", bufs=2))

        qf = singles.tile([128, Q], FP16)
        nc.sync.dma_start(qf[0:10, :], qf1_d[:])
        nc.sync.dma_start(qf[32:42, :], qf1_d[:])
        nc.sync.dma_start(qf[64:72, :], qf2_d[:])
        nc.sync.dma_start(qf[96:104, :], qf2_d[:])
        jw = N // JF_SPLIT
        jfs = []
        for t in range(JF_SPLIT):
            jft = singles.tile([128, jw], FP16, tag=f"jf{t}")
            cs = slice(t * jw, (t + 1) * jw)
            nc.sync.dma_start(jft[0:10, :], jf1_d[:, cs])
            nc.sync.dma_start(jft[32:42, :], jf1_d[:, cs])
            nc.sync.dma_start(jft[64:72, :], jf2_d[:, cs])
            nc.sync.dma_start(jft[96:104, :], jf2_d[:, cs])
            jfs.append(jft)
        # single-writer accumulator stripes; final math happens on host
        s_all = singles.tile([128, ITILES * NJC], F32)
        c_all = singles.tile([128, ITILES * NJP], F32)

        for it in range(ITILES):
            l1a = qf[0:10, bass.ts(it, 128)]
            l1b = qf[32:42, bass.ts(it, 128)]
            l2a = qf[64:72, bass.ts(it, 128)]
            l2b = qf[96:104, bass.ts(it, 128)]
            for p in range(NJP):
                g2i = it * NJP + p
                sd = sdp.tile([128, JPAIR], FP16, tag="sd")
                for h in range(2):
                    jc = p * 2 + h
                    gi = it * NJC + jc
                    g1 = psum.tile([128, JCHUNK], F32, tag="g1")
                    g2 = psum.tile([128, JCHUNK], F32, tag="g2")
                    c0 = jc * JCHUNK
                    c1 = c0 + 512
                    t0, t1 = jfs[c0 // jw], jfs[c1 // jw]
                    r0, r1 = c0 % jw, c1 % jw
                    nc.tensor.matmul(g1[:, 0:512], l1a, t0[0:10, r0:r0 + 512],
                                     tile_position=(0, 0))
                    nc.tensor.matmul(g1[:, 512:1024], l1b,
                                     t1[32:42, r1:r1 + 512],
                                     tile_position=(32, 0))
                    nc.tensor.matmul(g2[:, 0:512], l2a, t0[64:72, r0:r0 + 512],
                                     tile_position=(64, 0))
                    nc.tensor.matmul(g2[:, 512:1024], l2b,
                                     t1[96:104, r1:r1 + 512],
                                     tile_position=(96, 0))
                    dist = dsp.tile([128, JCHUNK], FP16, tag="dist")
                    nc.scalar.activation(dist[:], g1[:], ACTF.Sqrt)
                    hs = slice(h * JCHUNK, (h + 1) * JCHUNK)
                    nc.vector._custom_dve(masked_sd, out=sd[:, hs],
                                          in0=g2[:], in1=dist[:],
                                          accum_out=s_all[:, gi:gi + 1])
                if CNT_SCHED[g2i] == 'v':
                    mk = scr.tile([128, JPAIR], FP16, tag="mkv")
                    nc.vector.tensor_scalar(
                        out=mk[:], in0=sd[:], scalar1=0.0, scalar2=0.0,
                        op0=AluOpType.is_gt, op1=AluOpType.add,
                        accum_out=c_all[:, g2i:g2i + 1])
                else:
                    mk = scr.tile([128, JPAIR], FP8, tag="mka")
                    nc.scalar.activation(mk[:], sd[:], ACTF.Sign,
                                         accum_out=c_all[:, g2i:g2i + 1])

        nc.sync.dma_start(os_d[:], s_all[:])
        nc.sync.dma_start(oc_d[:], c_all[:])

    nc.compile()
    return nc


_CACHED_NC = None


def _get_nc():
    global _CACHED_NC
    if _CACHED_NC is None:
        _CACHED_NC = _build_graph()
    return _CACHED_NC


def _prep_inputs(past_ped_positions, ped_positions, indexes, all_radii):
    pos = np.asarray(ped_positions, np.float64)
    past = np.asarray(past_ped_positions, np.float64)
    v = pos - past
    vn = np.hypot(v[:, 0], v[:, 1])
    safe = np.where(vn > 0, vn, 1.0)
    ux = np.where(vn > 0, v[:, 0] / safe, 1.0)
    uy = np.where(vn > 0, v[:, 1] / safe, 0.0)

    px, py = pos[:, 0], pos[:, 1]
    nsq = px * px + py * py
    px_h, px_l = _split(px)
    py_h, py_l = _split(py)
    nsq_h, nsq_l = _split(nsq)
    ones = np.ones(N)
    jf1 = np.stack([px_h, px_l, px_h, py_h, py_l, py_h, ones, ones,
                    nsq_h, nsq_l]).astype(_F16)
    jf2 = jf1[0:8].copy()

    a = ux / COS_HALF
    b = uy / COS_HALF
    w = (ux * px + uy * py) / COS_HALF
    a_h, a_l = _split(a)
    b_h, b_l = _split(b)
    w_h, w_l = _split(w)
    nq_h, nq_l = _split(nsq + EPS)
    qf1_full = np.stack([-2 * px_h, -2 * px_h, -2 * px_l,
                         -2 * py_h, -2 * py_h, -2 * py_l,
                         nq_h, nq_l, ones, ones])  # [10, N]
    qf2_full = np.stack([a_h, a_h, a_l, b_h, b_h, b_l, -w_h, -w_l])  # [8, N]

    # column c of per-core qf holds local query (c % 128) * ITILES + c // 128
    cidx = np.arange(Q)
    perm = (cidx % 128) * ITILES + cidx // 128

    in_maps = []
    for k in range(NCORES):
        sl = slice(k * Q, (k + 1) * Q)
        qf1_core = qf1_full[:, sl][:, perm].astype(_F16)
        qf2_core = qf2_full[:, sl][:, perm].astype(_F16)
        in_maps.append({"qf1": qf1_core, "qf2": qf2_core, "jf1": jf1,
                        "jf2": jf2})
    return in_maps


def _host_epilogue(res_core, idxf_core, radii_core):
    """Accumulator stripes -> [1024] final radii for one core.
    idxf_core/radii_core are [128, ITILES] (local query q = p*ITILES + it)."""
    s = np.asarray(res_core["out_s"], np.float64).reshape(
        128, ITILES, NJC).sum(2)
    c = np.asarray(res_core["out_c"], np.float64).reshape(
        128, ITILES, NJP).sum(2)
    mean = (s / np.maximum(c, 1.0)).astype(np.float32)
    r = np.clip(mean * np.float32(SLOPE) + np.float32(OFFS), MIN_R, MAX_R)
    fin = radii_core + idxf_core * (r - radii_core)
    return fin.astype(np.float32).reshape(Q)


def kernel(past_ped_positions, ped_positions, indexes, all_radii,
           _trace=False, _trace_kwargs=None):
    nc = _get_nc()
    in_maps = _prep_inputs(past_ped_positions, ped_positions, indexes,
                           all_radii)
    kw = {}
    if _trace:
        kw = {"trace": True}
        if _trace_kwargs:
            kw.update(_trace_kwargs)
    res = run_bass_kernel_spmd(nc, in_maps, list(range(NCORES)), **kw)
    idxf = np.asarray(indexes).astype(np.float32)
    radii = np.asarray(all_radii, np.float32)
    outs = []
    for k in range(NCORES):
        sl = slice(k * Q, (k + 1) * Q)
        outs.append(_host_epilogue(res.results[k],
                                   idxf[sl].reshape(128, ITILES),
                                   radii[sl].reshape(128, ITILES)))
    out = np.concatenate(outs)
    if _trace:
        kernel.last_results = res
    return out


# revision 11
# speedup vs baseline: 1.0680x; 1.0680x over previous
"""Trainium2 Bass kernel for ArcShapeRadiusConfigVisibleNeighDist.

For each pedestrian i (N=8192):
  heading u_i = normalize(pos_i - past_i)
  over all j: dist_ij = |pos_j - pos_i|, visible iff angle(pos_j-pos_i, u_i)
  in [-35deg, 35deg) and j != i. Output = affine(clip(mean visible dist)).

Reformulation (no atan2): visible <=> dot/c > dist. sq and dot/c are
K-small matmuls on the TensorEngine with fp16 hi/lo split features. The
4 matmuls per chunk go to the 4 PE row-quads (tile_position 0/32/64/96)
so they stream concurrently. (Note: a DVE op may read at most ONE PSUM
operand, so the select must compare against SBUF dist, not PSUM sq.)

Per 128-query x 1024-j chunk:
  PE:   g1 = sq+eps, g2 = dot/c   (PSUM fp32, 2x512 each)
  ACT:  dist = sqrt(g1) -> fp16 SBUF
  DVE:  sd = select(g2 > dist, dist, 0) -> fp16, accum_out -> s (custom)
Per 2048-j pair of chunks:
  count: cnt = sum(sd > 0): DVE tensor_scalar+accum or ACT Sign+accum,
  per CNT_SCHED to balance engine load.
Epilogue (host): r = clip(slope*(s/max(cnt,1)) + off, 0.5, 4.0); indexes sel.

Sharding: core k owns queries [k*1024, (k+1)*1024), full j set.
"""

import numpy as np

import concourse.bass as bass
import concourse.bacc as bacc
import concourse.mybir as mybir
import concourse.tile as tile
from contextlib import ExitStack
from concourse.alu_op_type import AluOpType
from concourse.bass_utils import run_bass_kernel_spmd
from concourse.dve_uop import DveOpSpec
import concourse.dve_ops as dvo
from concourse.dve_ops import Spec, Src0, Src1, Zero, select, lower, has_src1
from concourse.dve_ops import AluOp as SAluOp
from concourse.dve_spec import sq as spec_sq

N = 8192
NCORES = 8
Q = N // NCORES            # 1024 queries per core
ITILES = Q // 128          # 8 partition tiles of queries
JCHUNK = 1024
JPAIR = 2 * JCHUNK         # 2048: sqrt/count granularity
NJP = N // JPAIR           # 4 j-pairs per i-tile
NJC = N // JCHUNK          # 8 j-chunks per i-tile
EPS = 0.005                # sq guard: keeps diag excluded, sqrt input > 0
COS_HALF = float(np.cos(70.0 * np.pi / 180.0 / 2.0))
MIN_R, MAX_R = 0.5, 4.0
MIN_D, MAX_D = 0.2, 5.0
SLOPE = (MAX_R - MIN_R) / (MAX_D - MIN_D)
OFFS = MIN_R - MIN_D * SLOPE

F32 = mybir.dt.float32
FP16 = mybir.dt.float16
FP8 = mybir.dt.float8e4
ACTF = mybir.ActivationFunctionType
_F16 = np.float16

# count-pass engine per pair (it * NJP + p): 'v' DVE tensor_scalar+accum,
# 'a' ACT Sign+accum. Tuned so DVE ~= ACT total busy.
N_V = 13                   # of ITILES*NJP = 32 pairs
CNT_SCHED = [('v' if (i * 19) % 32 < N_V else 'a') for i in range(ITILES * NJP)]
JF_SPLIT = 4               # jf DMA'd as column tiles so first matmuls start early


def register_masked_sd():
    """Legacy v1 op (kept for probe compatibility): out = select(in0 > in1,
    in1, 0), accum_out = sum(out)."""
    name = "MASKED_SD_ANT"
    if name in dvo._SUB_OPCODE_FOR_NAME:
        return getattr(dvo, name)

    def _ref(in0, in1, s0, s1, imm2):
        b = np.where(in0.astype(np.float32) > in1, in1, 0.0).astype(np.float32)
        return b, b.reshape(b.shape[0], -1).sum(axis=-1, keepdims=True)

    spec = Spec(body=select(Src0 > Src1, Src1, Zero), accum=SAluOp.ADD,
                reference=_ref)
    return _register(name, spec)


def register_masked_sq():
    """out = select((in0 > 0) & (in0^2 > in1), in1, 0); no accum.
    in0 = dot/c, in1 = sq + eps: squared-domain visibility select."""
    name = "MASKED_SQ_ANT"
    if name in dvo._SUB_OPCODE_FOR_NAME:
        return getattr(dvo, name)

    def _ref(in0, in1, s0, s1, imm2):
        a = in0.astype(np.float32)
        b = in1.astype(np.float32)
        return np.where((a > 0) & (a * a > b), b, 0.0).astype(np.float32)

    spec = Spec(body=select((Src0 > Zero) & (spec_sq(Src0) > Src1), Src1, Zero),
                reference=_ref)
    return _register(name, spec)


def _register(name, spec):
    """Runtime-register a custom DVE op. The per-NEFF uop table is generated
    from OPS, so appending at runtime is sufficient."""
    row = max(dvo._SUB_OPCODE_FOR_NAME.values()) + 1
    assert row < 0x20
    dvo._SUB_OPCODE_FOR_NAME[name] = row
    op = dvo.DveOp(name, spec, subdim=False, uops_sha={})
    for ver in ("v3", "v4"):
        s = DveOpSpec(name=name, opcode=row, uops=lower(spec, ver=ver),
                      rd1_en=has_src1(spec))
        op.uops_sha[ver] = s.sha(ver)
    dvo.OPS.append(op)
    dvo.CUSTOM_DVE_SPECS[name] = spec
    setattr(dvo, name, op)
    return op


def _split(x):
    """Split f64 array into fp16 hi + fp16 lo (as f64 of exact fp16 values)."""
    h = x.astype(_F16).astype(np.float64)
    l = (x - h).astype(_F16).astype(np.float64)
    return h, l


def _build_graph():
    masked_sd = register_masked_sd()
    nc = bacc.Bacc("TRN2", target_bir_lowering=False, debug=False,
                   num_devices=NCORES)
    # feature rows: G1 at partitions 0-9 and 32-41 (quads 0,1); G2 at
    # 64-71 and 96-103 (quads 2,3) -> 4 concurrent matmul streams.
    qf1_d = nc.dram_tensor("qf1", [10, Q], FP16, kind="ExternalInput")
    qf2_d = nc.dram_tensor("qf2", [8, Q], FP16, kind="ExternalInput")
    jf1_d = nc.dram_tensor("jf1", [10, N], FP16, kind="ExternalInput")
    jf2_d = nc.dram_tensor("jf2", [8, N], FP16, kind="ExternalInput")
    os_d = nc.dram_tensor("out_s", [128, ITILES * NJC], F32,
                          kind="ExternalOutput")
    oc_d = nc.dram_tensor("out_c", [128, ITILES * NJP], F32,
                          kind="ExternalOutput")

    with tile.TileContext(nc) as tc, ExitStack() as ctx:
        singles = ctx.enter_context(tc.tile_pool(name="singles", bufs=1))
        psum = ctx.enter_context(tc.tile_pool(name="psum", bufs=2, space="PSUM"))
        dsp = ctx.enter_context(tc.tile_pool(name="dsp", bufs=3))
        sdp = ctx.enter_context(tc.tile_pool(name="sdp", bufs=3))
        scr = ctx.enter_context(tc.tile_pool(name="scr", bufs=2))

        qf = singles.tile([128, Q], FP16)
        nc.sync.dma_start(qf[0:10, :], qf1_d[:])
        nc.sync.dma_start(qf[32:42, :], qf1_d[:])
        nc.sync.dma_start(qf[64:72, :], qf2_d[:])
        nc.sync.dma_start(qf[96:104, :], qf2_d[:])
        jw = N // JF_SPLIT
        jfs = []
        for t in range(JF_SPLIT):
            jft = singles.tile([128, jw], FP16, tag=f"jf{t}")
            cs = slice(t * jw, (t + 1) * jw)
            nc.sync.dma_start(jft[0:10, :], jf1_d[:, cs])
            nc.sync.dma_start(jft[32:42, :], jf1_d[:, cs])
            nc.sync.dma_start(jft[64:72, :], jf2_d[:, cs])
            nc.sync.dma_start(jft[96:104, :], jf2_d[:, cs])
            jfs.append(jft)
        # single-writer accumulator stripes; final math happens on host
        s_all = singles.tile([128, ITILES * NJC], F32)
        c_all = singles.tile([128, ITILES * NJP], F32)

        def emit_count(g2i, sd):
            """Count pass for one finished sd pair (deferred one pair so the
            in-order ACT/DVE queues never stall on a cross-engine dep)."""
            if CNT_SCHED[g2i] == 'v':
                mk = scr.tile([128, JPAIR], FP16, tag="mkv")
                nc.vector.tensor_scalar(
                    out=mk[:], in0=sd[:], scalar1=0.0, scalar2=0.0,
                    op0=AluOpType.is_gt, op1=AluOpType.add,
                    accum_out=c_all[:, g2i:g2i + 1])
            else:
                mk = scr.tile([128, JPAIR], FP8, tag="mka")
                nc.scalar.activation(mk[:], sd[:], ACTF.Sign,
                                     accum_out=c_all[:, g2i:g2i + 1])

        pending = []
        for it in range(ITILES):
            l1a = qf[0:10, bass.ts(it, 128)]
            l1b = qf[32:42, bass.ts(it, 128)]
            l2a = qf[64:72, bass.ts(it, 128)]
            l2b = qf[96:104, bass.ts(it, 128)]
            for p in range(NJP):
                g2i = it * NJP + p
                sd = sdp.tile([128, JPAIR], FP16, tag="sd")
                for h in range(2):
                    jc = p * 2 + h
                    gi = it * NJC + jc
                    g1 = psum.tile([128, JCHUNK], F32, tag="g1")
                    g2 = psum.tile([128, JCHUNK], F32, tag="g2")
                    c0 = jc * JCHUNK
                    c1 = c0 + 512
                    t0, t1 = jfs[c0 // jw], jfs[c1 // jw]
                    r0, r1 = c0 % jw, c1 % jw
                    nc.tensor.matmul(g1[:, 0:512], l1a, t0[0:10, r0:r0 + 512],
                                     tile_position=(0, 0))
                    nc.tensor.matmul(g1[:, 512:1024], l1b,
                                     t1[32:42, r1:r1 + 512],
                                     tile_position=(32, 0))
                    nc.tensor.matmul(g2[:, 0:512], l2a, t0[64:72, r0:r0 + 512],
                                     tile_position=(64, 0))
                    nc.tensor.matmul(g2[:, 512:1024], l2b,
                                     t1[96:104, r1:r1 + 512],
                                     tile_position=(96, 0))
                    dist = dsp.tile([128, JCHUNK], FP16, tag="dist")
                    nc.scalar.activation(dist[:], g1[:], ACTF.Sqrt)
                    hs = slice(h * JCHUNK, (h + 1) * JCHUNK)
                    nc.vector._custom_dve(masked_sd, out=sd[:, hs],
                                          in0=g2[:], in1=dist[:],
                                          accum_out=s_all[:, gi:gi + 1])
                pending.append((g2i, sd))
                if len(pending) > 1:
                    emit_count(*pending.pop(0))
        for item in pending:
            emit_count(*item)

        nc.sync.dma_start(os_d[:], s_all[:])
        nc.sync.dma_start(oc_d[:], c_all[:])

    nc.compile()
    return nc


_CACHED_NC = None


def _get_nc():
    global _CACHED_NC
    if _CACHED_NC is None:
        _CACHED_NC = _build_graph()
    return _CACHED_NC


def _prep_inputs(past_ped_positions, ped_positions, indexes, all_radii):
    pos = np.asarray(ped_positions, np.float64)
    past = np.asarray(past_ped_positions, np.float64)
    v = pos - past
    vn = np.hypot(v[:, 0], v[:, 1])
    safe = np.where(vn > 0, vn, 1.0)
    ux = np.where(vn > 0, v[:, 0] / safe, 1.0)
    uy = np.where(vn > 0, v[:, 1] / safe, 0.0)

    px, py = pos[:, 0], pos[:, 1]
    nsq = px * px + py * py
    px_h, px_l = _split(px)
    py_h, py_l = _split(py)
    nsq_h, nsq_l = _split(nsq)
    ones = np.ones(N)
    jf1 = np.stack([px_h, px_l, px_h, py_h, py_l, py_h, ones, ones,
                    nsq_h, nsq_l]).astype(_F16)
    jf2 = jf1[0:8].copy()

    a = ux / COS_HALF
    b = uy / COS_HALF
    w = (ux * px + uy * py) / COS_HALF
    a_h, a_l = _split(a)
    b_h, b_l = _split(b)
    w_h, w_l = _split(w)
    nq_h, nq_l = _split(nsq + EPS)
    qf1_full = np.stack([-2 * px_h, -2 * px_h, -2 * px_l,
                         -2 * py_h, -2 * py_h, -2 * py_l,
                         nq_h, nq_l, ones, ones])  # [10, N]
    qf2_full = np.stack([a_h, a_h, a_l, b_h, b_h, b_l, -w_h, -w_l])  # [8, N]

    # column c of per-core qf holds local query (c % 128) * ITILES + c // 128
    cidx = np.arange(Q)
    perm = (cidx % 128) * ITILES + cidx // 128

    in_maps = []
    for k in range(NCORES):
        sl = slice(k * Q, (k + 1) * Q)
        qf1_core = qf1_full[:, sl][:, perm].astype(_F16)
        qf2_core = qf2_full[:, sl][:, perm].astype(_F16)
        in_maps.append({"qf1": qf1_core, "qf2": qf2_core, "jf1": jf1,
                        "jf2": jf2})
    return in_maps


def _host_epilogue(res_core, idxf_core, radii_core):
    """Accumulator stripes -> [1024] final radii for one core.
    idxf_core/radii_core are [128, ITILES] (local query q = p*ITILES + it)."""
    s = np.asarray(res_core["out_s"], np.float64).reshape(
        128, ITILES, NJC).sum(2)
    c = np.asarray(res_core["out_c"], np.float64).reshape(
        128, ITILES, NJP).sum(2)
    mean = (s / np.maximum(c, 1.0)).astype(np.float32)
    r = np.clip(mean * np.float32(SLOPE) + np.float32(OFFS), MIN_R, MAX_R)
    fin = radii_core + idxf_core * (r - radii_core)
    return fin.astype(np.float32).reshape(Q)


def kernel(past_ped_positions, ped_positions, indexes, all_radii,
           _trace=False, _trace_kwargs=None):
    nc = _get_nc()
    in_maps = _prep_inputs(past_ped_positions, ped_positions, indexes,
                           all_radii)
    kw = {}
    if _trace:
        kw = {"trace": True}
        if _trace_kwargs:
            kw.update(_trace_kwargs)
    res = run_bass_kernel_spmd(nc, in_maps, list(range(NCORES)), **kw)
    idxf = np.asarray(indexes).astype(np.float32)
    radii = np.asarray(all_radii, np.float32)
    outs = []
    for k in range(NCORES):
        sl = slice(k * Q, (k + 1) * Q)
        outs.append(_host_epilogue(res.results[k],
                                   idxf[sl].reshape(128, ITILES),
                                   radii[sl].reshape(128, ITILES)))
    out = np.concatenate(outs)
    if _trace:
        kernel.last_results = res
    return out


# revision 13
# speedup vs baseline: 1.0753x; 1.0068x over previous
"""Trainium2 Bass kernel for ArcShapeRadiusConfigVisibleNeighDist.

For each pedestrian i (N=8192):
  heading u_i = normalize(pos_i - past_i)
  over all j: dist_ij = |pos_j - pos_i|, visible iff angle(pos_j-pos_i, u_i)
  in [-35deg, 35deg) and j != i. Output = affine(clip(mean visible dist)).

Reformulation (no atan2): visible <=> dot/c > dist. sq and dot/c are
K-small matmuls on the TensorEngine with fp16 hi/lo split features. The
4 matmuls per chunk go to the 4 PE row-quads (tile_position 0/32/64/96)
so they stream concurrently. (Note: a DVE op may read at most ONE PSUM
operand, so the select must compare against SBUF dist, not PSUM sq.)

Per 128-query x 1024-j chunk:
  PE:   g1 = sq+eps, g2 = dot/c   (PSUM fp32, 2x512 each)
  ACT:  dist = sqrt(g1) -> fp16 SBUF
  DVE:  sd = select(g2 > dist, dist, 0) -> fp16, accum_out -> s (custom)
Per 2048-j pair of chunks:
  count: cnt = sum(sd > 0): DVE tensor_scalar+accum or ACT Sign+accum,
  per CNT_SCHED to balance engine load.
Epilogue (host): r = clip(slope*(s/max(cnt,1)) + off, 0.5, 4.0); indexes sel.

Sharding: core k owns queries [k*1024, (k+1)*1024), full j set.
"""

import numpy as np

import concourse.bass as bass
import concourse.bacc as bacc
import concourse.mybir as mybir
import concourse.tile as tile
from contextlib import ExitStack
from concourse.alu_op_type import AluOpType
from concourse.bass_utils import run_bass_kernel_spmd
from concourse.dve_uop import DveOpSpec
import concourse.dve_ops as dvo
from concourse.dve_ops import Spec, Src0, Src1, Zero, select, lower, has_src1
from concourse.dve_ops import AluOp as SAluOp
from concourse.dve_spec import sq as spec_sq

N = 8192
NCORES = 8
Q = N // NCORES            # 1024 queries per core
ITILES = Q // 128          # 8 partition tiles of queries
JCHUNK = 1024
JPAIR = 2 * JCHUNK         # 2048: sqrt/count granularity
NJP = N // JPAIR           # 4 j-pairs per i-tile
NJC = N // JCHUNK          # 8 j-chunks per i-tile
EPS = 0.005                # sq guard: keeps diag excluded, sqrt input > 0
COS_HALF = float(np.cos(70.0 * np.pi / 180.0 / 2.0))
MIN_R, MAX_R = 0.5, 4.0
MIN_D, MAX_D = 0.2, 5.0
SLOPE = (MAX_R - MIN_R) / (MAX_D - MIN_D)
OFFS = MIN_R - MIN_D * SLOPE

F32 = mybir.dt.float32
FP16 = mybir.dt.float16
FP8 = mybir.dt.float8e4
ACTF = mybir.ActivationFunctionType
_F16 = np.float16

# count-pass engine per pair (it * NJP + p): 'v' DVE tensor_scalar+accum,
# 'a' ACT Sign+accum. Tuned so DVE ~= ACT total busy.
N_V = 13                   # of ITILES*NJP = 32 pairs
CNT_SCHED = [('v' if (i * 19) % 32 < N_V else 'a') for i in range(ITILES * NJP)]
JF_SPLIT = 4               # jf DMA'd as column tiles so first matmuls start early


def register_masked_sd():
    """Legacy v1 op (kept for probe compatibility): out = select(in0 > in1,
    in1, 0), accum_out = sum(out)."""
    name = "MASKED_SD_ANT"
    if name in dvo._SUB_OPCODE_FOR_NAME:
        return getattr(dvo, name)

    def _ref(in0, in1, s0, s1, imm2):
        b = np.where(in0.astype(np.float32) > in1, in1, 0.0).astype(np.float32)
        return b, b.reshape(b.shape[0], -1).sum(axis=-1, keepdims=True)

    spec = Spec(body=select(Src0 > Src1, Src1, Zero), accum=SAluOp.ADD,
                reference=_ref)
    return _register(name, spec)


def register_masked_sq():
    """out = select((in0 > 0) & (in0^2 > in1), in1, 0); no accum.
    in0 = dot/c, in1 = sq + eps: squared-domain visibility select."""
    name = "MASKED_SQ_ANT"
    if name in dvo._SUB_OPCODE_FOR_NAME:
        return getattr(dvo, name)

    def _ref(in0, in1, s0, s1, imm2):
        a = in0.astype(np.float32)
        b = in1.astype(np.float32)
        return np.where((a > 0) & (a * a > b), b, 0.0).astype(np.float32)

    spec = Spec(body=select((Src0 > Zero) & (spec_sq(Src0) > Src1), Src1, Zero),
                reference=_ref)
    return _register(name, spec)


def _register(name, spec):
    """Runtime-register a custom DVE op. The per-NEFF uop table is generated
    from OPS, so appending at runtime is sufficient."""
    row = max(dvo._SUB_OPCODE_FOR_NAME.values()) + 1
    assert row < 0x20
    dvo._SUB_OPCODE_FOR_NAME[name] = row
    op = dvo.DveOp(name, spec, subdim=False, uops_sha={})
    for ver in ("v3", "v4"):
        s = DveOpSpec(name=name, opcode=row, uops=lower(spec, ver=ver),
                      rd1_en=has_src1(spec))
        op.uops_sha[ver] = s.sha(ver)
    dvo.OPS.append(op)
    dvo.CUSTOM_DVE_SPECS[name] = spec
    setattr(dvo, name, op)
    return op


def _split(x):
    """Split f64 array into fp16 hi + fp16 lo (as f64 of exact fp16 values)."""
    h = x.astype(_F16).astype(np.float64)
    l = (x - h).astype(_F16).astype(np.float64)
    return h, l


def _build_graph():
    masked_sd = register_masked_sd()
    nc = bacc.Bacc("TRN2", target_bir_lowering=False, debug=False,
                   num_devices=NCORES)
    # feature rows: G1 at partitions 0-9 and 32-41 (quads 0,1); G2 at
    # 64-71 and 96-103 (quads 2,3) -> 4 concurrent matmul streams.
    qf1_d = nc.dram_tensor("qf1", [10, Q], FP16, kind="ExternalInput")
    qf2_d = nc.dram_tensor("qf2", [8, Q], FP16, kind="ExternalInput")
    jf1_d = nc.dram_tensor("jf1", [10, N], FP16, kind="ExternalInput")
    jf2_d = nc.dram_tensor("jf2", [8, N], FP16, kind="ExternalInput")
    os_d = nc.dram_tensor("out_s", [128, ITILES * NJC], F32,
                          kind="ExternalOutput")
    oc_d = nc.dram_tensor("out_c", [128, ITILES * NJP], F32,
                          kind="ExternalOutput")

    with tile.TileContext(nc) as tc, ExitStack() as ctx:
        singles = ctx.enter_context(tc.tile_pool(name="singles", bufs=1))
        psum = ctx.enter_context(tc.tile_pool(name="psum", bufs=2, space="PSUM"))
        dsp = ctx.enter_context(tc.tile_pool(name="dsp", bufs=4))
        sdp = ctx.enter_context(tc.tile_pool(name="sdp", bufs=4))
        scr = ctx.enter_context(tc.tile_pool(name="scr", bufs=3))

        # force the Sqrt/Sign ACT table set resident at t~0 (the lazy load
        # otherwise lands in front of the first real sqrt, ~6us into the run)
        warm = singles.tile([128, 1], F32, tag="warm")
        nc.vector.memset(warm[:], 1.0)
        nc.scalar.activation(warm[:], warm[:], ACTF.Sqrt)

        qf = singles.tile([128, Q], FP16)
        jw = N // JF_SPLIT
        jfs = []
        for t in range(JF_SPLIT):
            jft = singles.tile([128, jw], FP16, tag=f"jf{t}")
            jfs.append(jft)
        # first-needed rows first: the opening matmuls read qf G1/G2 rows and
        # jf tile 0; later jf tiles may land behind the compute front
        nc.sync.dma_start(qf[0:10, :], qf1_d[:])
        nc.sync.dma_start(qf[32:42, :], qf1_d[:])
        nc.sync.dma_start(qf[64:72, :], qf2_d[:])
        nc.sync.dma_start(qf[96:104, :], qf2_d[:])
        for t in range(JF_SPLIT):
            jft = jfs[t]
            cs = slice(t * jw, (t + 1) * jw)
            nc.sync.dma_start(jft[0:10, :], jf1_d[:, cs])
            nc.sync.dma_start(jft[32:42, :], jf1_d[:, cs])
            nc.sync.dma_start(jft[64:72, :], jf2_d[:, cs])
            nc.sync.dma_start(jft[96:104, :], jf2_d[:, cs])
        # single-writer accumulator stripes; final math happens on host
        s_all = singles.tile([128, ITILES * NJC], F32)
        c_all = singles.tile([128, ITILES * NJP], F32)

        def emit_count(g2i, sd):
            """Count pass for one finished sd pair (deferred one pair so the
            in-order ACT/DVE queues never stall on a cross-engine dep)."""
            if CNT_SCHED[g2i] == 'v':
                mk = scr.tile([128, JPAIR], FP16, tag="mkv")
                nc.vector.tensor_scalar(
                    out=mk[:], in0=sd[:], scalar1=0.0, scalar2=0.0,
                    op0=AluOpType.is_gt, op1=AluOpType.add,
                    accum_out=c_all[:, g2i:g2i + 1])
            else:
                mk = scr.tile([128, JPAIR], FP8, tag="mka")
                nc.scalar.activation(mk[:], sd[:], ACTF.Sign,
                                     accum_out=c_all[:, g2i:g2i + 1])

        pending = []
        for it in range(ITILES):
            l1a = qf[0:10, bass.ts(it, 128)]
            l1b = qf[32:42, bass.ts(it, 128)]
            l2a = qf[64:72, bass.ts(it, 128)]
            l2b = qf[96:104, bass.ts(it, 128)]
            for p in range(NJP):
                g2i = it * NJP + p
                sd = sdp.tile([128, JPAIR], FP16, tag="sd")
                for h in range(2):
                    jc = p * 2 + h
                    gi = it * NJC + jc
                    g1 = psum.tile([128, JCHUNK], F32, tag="g1")
                    g2 = psum.tile([128, JCHUNK], F32, tag="g2")
                    c0 = jc * JCHUNK
                    c1 = c0 + 512
                    t0, t1 = jfs[c0 // jw], jfs[c1 // jw]
                    r0, r1 = c0 % jw, c1 % jw
                    nc.tensor.matmul(g1[:, 0:512], l1a, t0[0:10, r0:r0 + 512],
                                     tile_position=(0, 0))
                    nc.tensor.matmul(g1[:, 512:1024], l1b,
                                     t1[32:42, r1:r1 + 512],
                                     tile_position=(32, 0))
                    nc.tensor.matmul(g2[:, 0:512], l2a, t0[64:72, r0:r0 + 512],
                                     tile_position=(64, 0))
                    nc.tensor.matmul(g2[:, 512:1024], l2b,
                                     t1[96:104, r1:r1 + 512],
                                     tile_position=(96, 0))
                    dist = dsp.tile([128, JCHUNK], FP16, tag="dist")
                    nc.scalar.activation(dist[:], g1[:], ACTF.Sqrt)
                    hs = slice(h * JCHUNK, (h + 1) * JCHUNK)
                    nc.vector._custom_dve(masked_sd, out=sd[:, hs],
                                          in0=g2[:], in1=dist[:],
                                          accum_out=s_all[:, gi:gi + 1])
                pending.append((g2i, sd))
                if len(pending) > 1:
                    emit_count(*pending.pop(0))
        for item in pending:
            emit_count(*item)

        nc.sync.dma_start(os_d[:], s_all[:])
        nc.sync.dma_start(oc_d[:], c_all[:])

    nc.compile()
    return nc


_CACHED_NC = None


def _get_nc():
    global _CACHED_NC
    if _CACHED_NC is None:
        _CACHED_NC = _build_graph()
    return _CACHED_NC


def _prep_inputs(past_ped_positions, ped_positions, indexes, all_radii):
    pos = np.asarray(ped_positions, np.float64)
    past = np.asarray(past_ped_positions, np.float64)
    v = pos - past
    vn = np.hypot(v[:, 0], v[:, 1])
    safe = np.where(vn > 0, vn, 1.0)
    ux = np.where(vn > 0, v[:, 0] / safe, 1.0)
    uy = np.where(vn > 0, v[:, 1] / safe, 0.0)

    px, py = pos[:, 0], pos[:, 1]
    nsq = px * px + py * py
    px_h, px_l = _split(px)
    py_h, py_l = _split(py)
    nsq_h, nsq_l = _split(nsq)
    ones = np.ones(N)
    jf1 = np.stack([px_h, px_l, px_h, py_h, py_l, py_h, ones, ones,
                    nsq_h, nsq_l]).astype(_F16)
    jf2 = jf1[0:8].copy()

    a = ux / COS_HALF
    b = uy / COS_HALF
    w = (ux * px + uy * py) / COS_HALF
    a_h, a_l = _split(a)
    b_h, b_l = _split(b)
    w_h, w_l = _split(w)
    nq_h, nq_l = _split(nsq + EPS)
    qf1_full = np.stack([-2 * px_h, -2 * px_h, -2 * px_l,
                         -2 * py_h, -2 * py_h, -2 * py_l,
                         nq_h, nq_l, ones, ones])  # [10, N]
    qf2_full = np.stack([a_h, a_h, a_l, b_h, b_h, b_l, -w_h, -w_l])  # [8, N]

    # column c of per-core qf holds local query (c % 128) * ITILES + c // 128
    cidx = np.arange(Q)
    perm = (cidx % 128) * ITILES + cidx // 128

    in_maps = []
    for k in range(NCORES):
        sl = slice(k * Q, (k + 1) * Q)
        qf1_core = qf1_full[:, sl][:, perm].astype(_F16)
        qf2_core = qf2_full[:, sl][:, perm].astype(_F16)
        in_maps.append({"qf1": qf1_core, "qf2": qf2_core, "jf1": jf1,
                        "jf2": jf2})
    return in_maps


def _host_epilogue(res_core, idxf_core, radii_core):
    """Accumulator stripes -> [1024] final radii for one core.
    idxf_core/radii_core are [128, ITILES] (local query q = p*ITILES + it)."""
    s = np.asarray(res_core["out_s"], np.float64).reshape(
        128, ITILES, NJC).sum(2)
    c = np.asarray(res_core["out_c"], np.float64).reshape(
        128, ITILES, NJP).sum(2)
    mean = (s / np.maximum(c, 1.0)).astype(np.float32)
    r = np.clip(mean * np.float32(SLOPE) + np.float32(OFFS), MIN_R, MAX_R)
    fin = radii_core + idxf_core * (r - radii_core)
    return fin.astype(np.float32).reshape(Q)


def kernel(past_ped_positions, ped_positions, indexes, all_radii,
           _trace=False, _trace_kwargs=None):
    nc = _get_nc()
    in_maps = _prep_inputs(past_ped_positions, ped_positions, indexes,
                           all_radii)
    kw = {}
    if _trace:
        kw = {"trace": True}
        if _trace_kwargs:
            kw.update(_trace_kwargs)
    res = run_bass_kernel_spmd(nc, in_maps, list(range(NCORES)), **kw)
    idxf = np.asarray(indexes).astype(np.float32)
    radii = np.asarray(all_radii, np.float32)
    outs = []
    for k in range(NCORES):
        sl = slice(k * Q, (k + 1) * Q)
        outs.append(_host_epilogue(res.results[k],
                                   idxf[sl].reshape(128, ITILES),
                                   radii[sl].reshape(128, ITILES)))
    out = np.concatenate(outs)
    if _trace:
        kernel.last_results = res
    return out


# revision 17
# speedup vs baseline: 1.0757x; 1.0004x over previous
"""Trainium2 Bass kernel for ArcShapeRadiusConfigVisibleNeighDist.

For each pedestrian i (N=8192):
  heading u_i = normalize(pos_i - past_i)
  over all j: dist_ij = |pos_j - pos_i|, visible iff angle(pos_j-pos_i, u_i)
  in [-35deg, 35deg) and j != i. Output = affine(clip(mean visible dist)).

Reformulation (no atan2): visible <=> dot/c > dist. sq and dot/c are
K-small matmuls on the TensorEngine with fp16 hi/lo split features. The
4 matmuls per chunk go to the 4 PE row-quads (tile_position 0/32/64/96)
so they stream concurrently. (Note: a DVE op may read at most ONE PSUM
operand, so the select must compare against SBUF dist, not PSUM sq.)

Per 128-query x 1024-j chunk:
  PE:   g1 = sq+eps, g2 = dot/c   (PSUM fp32, 2x512 each)
  ACT:  dist = sqrt(g1) -> fp16 SBUF
  DVE:  sd = select(g2 > dist, dist, 0) -> fp16, accum_out -> s (custom)
Per 2048-j pair of chunks:
  count: cnt = sum(sd > 0): DVE tensor_scalar+accum or ACT Sign+accum,
  per CNT_SCHED to balance engine load.
Epilogue (host): r = clip(slope*(s/max(cnt,1)) + off, 0.5, 4.0); indexes sel.

Sharding: core k owns queries [k*1024, (k+1)*1024), full j set.
"""

import numpy as np

import concourse.bass as bass
import concourse.bacc as bacc
import concourse.mybir as mybir
import concourse.tile as tile
from contextlib import ExitStack
from concourse.alu_op_type import AluOpType
from concourse.bass_utils import run_bass_kernel_spmd
from concourse.dve_uop import DveOpSpec
import concourse.dve_ops as dvo
from concourse.dve_ops import Spec, Src0, Src1, Zero, select, lower, has_src1
from concourse.dve_ops import AluOp as SAluOp
from concourse.dve_spec import sq as spec_sq

N = 8192
NCORES = 8
Q = N // NCORES            # 1024 queries per core
ITILES = Q // 128          # 8 partition tiles of queries
JCHUNK = 1024
JPAIR = 2 * JCHUNK         # 2048: sqrt/count granularity
NJP = N // JPAIR           # 4 j-pairs per i-tile
NJC = N // JCHUNK          # 8 j-chunks per i-tile
EPS = 0.005                # sq guard: keeps diag excluded, sqrt input > 0
COS_HALF = float(np.cos(70.0 * np.pi / 180.0 / 2.0))
MIN_R, MAX_R = 0.5, 4.0
MIN_D, MAX_D = 0.2, 5.0
SLOPE = (MAX_R - MIN_R) / (MAX_D - MIN_D)
OFFS = MIN_R - MIN_D * SLOPE

F32 = mybir.dt.float32
FP16 = mybir.dt.float16
FP8 = mybir.dt.float8e4
ACTF = mybir.ActivationFunctionType
_F16 = np.float16

# count-pass engine per pair (it * NJP + p): 'v' DVE tensor_scalar+accum,
# 'a' ACT Sign+accum. Tuned so DVE ~= ACT total busy.
N_V = 13                   # of ITILES*NJP = 32 pairs
CNT_SCHED = [('v' if (i * 19) % 32 < N_V else 'a') for i in range(ITILES * NJP)]
JF_SPLIT = 4               # jf DMA'd as column tiles so first matmuls start early


def register_masked_sd():
    """Legacy v1 op (kept for probe compatibility): out = select(in0 > in1,
    in1, 0), accum_out = sum(out)."""
    name = "MASKED_SD_ANT"
    if name in dvo._SUB_OPCODE_FOR_NAME:
        return getattr(dvo, name)

    def _ref(in0, in1, s0, s1, imm2):
        b = np.where(in0.astype(np.float32) > in1, in1, 0.0).astype(np.float32)
        return b, b.reshape(b.shape[0], -1).sum(axis=-1, keepdims=True)

    spec = Spec(body=select(Src0 > Src1, Src1, Zero), accum=SAluOp.ADD,
                reference=_ref)
    return _register(name, spec)


def register_masked_sq():
    """out = select((in0 > 0) & (in0^2 > in1), in1, 0); no accum.
    in0 = dot/c, in1 = sq + eps: squared-domain visibility select."""
    name = "MASKED_SQ_ANT"
    if name in dvo._SUB_OPCODE_FOR_NAME:
        return getattr(dvo, name)

    def _ref(in0, in1, s0, s1, imm2):
        a = in0.astype(np.float32)
        b = in1.astype(np.float32)
        return np.where((a > 0) & (a * a > b), b, 0.0).astype(np.float32)

    spec = Spec(body=select((Src0 > Zero) & (spec_sq(Src0) > Src1), Src1, Zero),
                reference=_ref)
    return _register(name, spec)


def _register(name, spec):
    """Runtime-register a custom DVE op. The per-NEFF uop table is generated
    from OPS, so appending at runtime is sufficient."""
    row = max(dvo._SUB_OPCODE_FOR_NAME.values()) + 1
    assert row < 0x20
    dvo._SUB_OPCODE_FOR_NAME[name] = row
    op = dvo.DveOp(name, spec, subdim=False, uops_sha={})
    for ver in ("v3", "v4"):
        s = DveOpSpec(name=name, opcode=row, uops=lower(spec, ver=ver),
                      rd1_en=has_src1(spec))
        op.uops_sha[ver] = s.sha(ver)
    dvo.OPS.append(op)
    dvo.CUSTOM_DVE_SPECS[name] = spec
    setattr(dvo, name, op)
    return op


def _split(x):
    """Split f64 array into fp16 hi + fp16 lo (as f64 of exact fp16 values)."""
    h = x.astype(_F16).astype(np.float64)
    l = (x - h).astype(_F16).astype(np.float64)
    return h, l


def _build_graph():
    masked_sd = register_masked_sd()
    nc = bacc.Bacc("TRN2", target_bir_lowering=False, debug=False,
                   num_devices=NCORES)
    # feature rows: G1 at partitions 0-9 and 32-41 (quads 0,1); G2 at
    # 64-71 and 96-103 (quads 2,3) -> 4 concurrent matmul streams.
    qf1_d = nc.dram_tensor("qf1", [10, Q], FP16, kind="ExternalInput")
    qf2_d = nc.dram_tensor("qf2", [8, Q], FP16, kind="ExternalInput")
    jf1_d = nc.dram_tensor("jf1", [10, N], FP16, kind="ExternalInput")
    jf2_d = nc.dram_tensor("jf2", [8, N], FP16, kind="ExternalInput")
    os_d = nc.dram_tensor("out_s", [128, ITILES * NJC], F32,
                          kind="ExternalOutput")
    oc_d = nc.dram_tensor("out_c", [128, ITILES * NJP], F32,
                          kind="ExternalOutput")

    with tile.TileContext(nc) as tc, ExitStack() as ctx:
        singles = ctx.enter_context(tc.tile_pool(name="singles", bufs=1))
        psum = ctx.enter_context(tc.tile_pool(name="psum", bufs=2, space="PSUM"))
        dsp = ctx.enter_context(tc.tile_pool(name="dsp", bufs=4))
        sdp = ctx.enter_context(tc.tile_pool(name="sdp", bufs=4))
        scr = ctx.enter_context(tc.tile_pool(name="scr", bufs=3))

        # force the Sqrt/Sign ACT table set resident at t~0 (the lazy load
        # otherwise lands in front of the first real sqrt, ~6us into the run)
        warm = singles.tile([128, 1], F32, tag="warm")
        nc.vector.memset(warm[:], 1.0)
        nc.scalar.activation(warm[:], warm[:], ACTF.Sqrt)

        qf = singles.tile([128, Q], FP16)
        jw = N // JF_SPLIT
        jfs = []
        for t in range(JF_SPLIT):
            jft = singles.tile([128, jw], FP16, tag=f"jf{t}")
            jfs.append(jft)
        # first-needed rows first: the opening matmuls read qf G1/G2 rows and
        # jf tile 0; later jf tiles may land behind the compute front
        nc.sync.dma_start(qf[0:10, :], qf1_d[:])
        nc.sync.dma_start(qf[32:42, :], qf1_d[:])
        nc.sync.dma_start(qf[64:72, :], qf2_d[:])
        nc.sync.dma_start(qf[96:104, :], qf2_d[:])
        for t in range(JF_SPLIT):
            jft = jfs[t]
            cs = slice(t * jw, (t + 1) * jw)
            nc.sync.dma_start(jft[0:10, :], jf1_d[:, cs])
            nc.sync.dma_start(jft[32:42, :], jf1_d[:, cs])
            nc.sync.dma_start(jft[64:72, :], jf2_d[:, cs])
            nc.sync.dma_start(jft[96:104, :], jf2_d[:, cs])
        # single-writer accumulator stripes; final math happens on host
        s_all = singles.tile([128, ITILES * NJC], F32)
        c_all = singles.tile([128, ITILES * NJP], F32)

        def emit_count(g2i, sd):
            """Count pass for one finished sd pair (deferred one pair so the
            in-order ACT/DVE queues never stall on a cross-engine dep)."""
            if CNT_SCHED[g2i] == 'v':
                mk = scr.tile([128, JPAIR], FP16, tag="mkv")
                nc.vector.tensor_scalar(
                    out=mk[:], in0=sd[:], scalar1=0.0, scalar2=0.0,
                    op0=AluOpType.is_gt, op1=AluOpType.add,
                    accum_out=c_all[:, g2i:g2i + 1])
            else:
                mk = scr.tile([128, JPAIR], FP8, tag="mka")
                nc.scalar.activation(mk[:], sd[:], ACTF.Sign,
                                     accum_out=c_all[:, g2i:g2i + 1])

        pending = []
        for it in range(ITILES):
            l1a = qf[0:10, bass.ts(it, 128)]
            l1b = qf[32:42, bass.ts(it, 128)]
            l2a = qf[64:72, bass.ts(it, 128)]
            l2b = qf[96:104, bass.ts(it, 128)]
            for p in range(NJP):
                g2i = it * NJP + p
                sd = sdp.tile([128, JPAIR], FP16, tag="sd")
                for h in range(2):
                    jc = p * 2 + h
                    gi = it * NJC + jc
                    g1 = psum.tile([128, JCHUNK], F32, tag="g1")
                    g2 = psum.tile([128, JCHUNK], F32, tag="g2")
                    c0 = jc * JCHUNK
                    c1 = c0 + 512
                    t0, t1 = jfs[c0 // jw], jfs[c1 // jw]
                    r0, r1 = c0 % jw, c1 % jw
                    nc.tensor.matmul(g1[:, 0:512], l1a, t0[0:10, r0:r0 + 512],
                                     tile_position=(0, 0))
                    nc.tensor.matmul(g1[:, 512:1024], l1b,
                                     t1[32:42, r1:r1 + 512],
                                     tile_position=(32, 0))
                    nc.tensor.matmul(g2[:, 0:512], l2a, t0[64:72, r0:r0 + 512],
                                     tile_position=(64, 0))
                    nc.tensor.matmul(g2[:, 512:1024], l2b,
                                     t1[96:104, r1:r1 + 512],
                                     tile_position=(96, 0))
                    dist = dsp.tile([128, JCHUNK], FP16, tag="dist")
                    nc.scalar.activation(dist[:], g1[:], ACTF.Sqrt)
                    hs = slice(h * JCHUNK, (h + 1) * JCHUNK)
                    nc.vector._custom_dve(masked_sd, out=sd[:, hs],
                                          in0=g2[:], in1=dist[:],
                                          accum_out=s_all[:, gi:gi + 1])
                pending.append((g2i, sd))
                if len(pending) > 1:
                    emit_count(*pending.pop(0))
        for item in pending:
            emit_count(*item)

        nc.sync.dma_start(os_d[:], s_all[:])
        nc.sync.dma_start(oc_d[:], c_all[:])

    nc.compile()
    return nc


_CACHED_NC = None


def _get_nc():
    global _CACHED_NC
    if _CACHED_NC is None:
        _CACHED_NC = _build_graph()
    return _CACHED_NC


def _prep_inputs(past_ped_positions, ped_positions, indexes, all_radii):
    pos = np.asarray(ped_positions, np.float64)
    past = np.asarray(past_ped_positions, np.float64)
    v = pos - past
    vn = np.hypot(v[:, 0], v[:, 1])
    safe = np.where(vn > 0, vn, 1.0)
    ux = np.where(vn > 0, v[:, 0] / safe, 1.0)
    uy = np.where(vn > 0, v[:, 1] / safe, 0.0)

    px, py = pos[:, 0], pos[:, 1]
    nsq = px * px + py * py
    px_h, px_l = _split(px)
    py_h, py_l = _split(py)
    nsq_h, nsq_l = _split(nsq)
    ones = np.ones(N)
    jf1 = np.stack([px_h, px_l, px_h, py_h, py_l, py_h, ones, ones,
                    nsq_h, nsq_l]).astype(_F16)
    jf2 = jf1[0:8].copy()

    a = ux / COS_HALF
    b = uy / COS_HALF
    w = (ux * px + uy * py) / COS_HALF
    a_h, a_l = _split(a)
    b_h, b_l = _split(b)
    w_h, w_l = _split(w)
    nq_h, nq_l = _split(nsq + EPS)
    qf1_full = np.stack([-2 * px_h, -2 * px_h, -2 * px_l,
                         -2 * py_h, -2 * py_h, -2 * py_l,
                         nq_h, nq_l, ones, ones])  # [10, N]
    qf2_full = np.stack([a_h, a_h, a_l, b_h, b_h, b_l, -w_h, -w_l])  # [8, N]

    # column c of per-core qf holds local query (c % 128) * ITILES + c // 128
    cidx = np.arange(Q)
    perm = (cidx % 128) * ITILES + cidx // 128

    in_maps = []
    for k in range(NCORES):
        sl = slice(k * Q, (k + 1) * Q)
        qf1_core = qf1_full[:, sl][:, perm].astype(_F16)
        qf2_core = qf2_full[:, sl][:, perm].astype(_F16)
        in_maps.append({"qf1": qf1_core, "qf2": qf2_core, "jf1": jf1,
                        "jf2": jf2})
    return in_maps


def _host_epilogue(res_core, idxf_core, radii_core):
    """Accumulator stripes -> [1024] final radii for one core.
    idxf_core/radii_core are [128, ITILES] (local query q = p*ITILES + it)."""
    s = np.asarray(res_core["out_s"], np.float64).reshape(
        128, ITILES, NJC).sum(2)
    c = np.asarray(res_core["out_c"], np.float64).reshape(
        128, ITILES, NJP).sum(2)
    mean = (s / np.maximum(c, 1.0)).astype(np.float32)
    r = np.clip(mean * np.float32(SLOPE) + np.float32(OFFS), MIN_R, MAX_R)
    fin = radii_core + idxf_core * (r - radii_core)
    return fin.astype(np.float32).reshape(Q)


def kernel(past_ped_positions, ped_positions, indexes, all_radii,
           _trace=False, _trace_kwargs=None):
    # note: the walrus LDWEIGHTS-dedupe pass (--enable-ldw-opt) would remove
    # the redundant same-quad weight reloads across j-chunks (each 512-col
    # matmul currently pays a ~219-cycle cold restart, 598ns vs ~216ns warm),
    # but it rejects this kernel's ldweights ("not compatible with LDW
    # optimization"), so the reloads stay.
    nc = _get_nc()
    in_maps = _prep_inputs(past_ped_positions, ped_positions, indexes,
                           all_radii)
    kw = {}
    if _trace:
        kw = {"trace": True}
        if _trace_kwargs:
            kw.update(_trace_kwargs)
    res = run_bass_kernel_spmd(nc, in_maps, list(range(NCORES)), **kw)
    idxf = np.asarray(indexes).astype(np.float32)
    radii = np.asarray(all_radii, np.float32)
    outs = []
    for k in range(NCORES):
        sl = slice(k * Q, (k + 1) * Q)
        outs.append(_host_epilogue(res.results[k],
                                   idxf[sl].reshape(128, ITILES),
                                   radii[sl].reshape(128, ITILES)))
    out = np.concatenate(outs)
    if _trace:
        kernel.last_results = res
    return out
